# revision 1
# baseline (speedup 1.0000x reference)
"""GCN forward (4-layer GCNConv + global mean-pool + linear) on 8 TRN2 cores.

Strategy (graph/dst-node data parallelism per the sharding hint):
  * Associativity: S @ (h W) == (S @ h) W  -> message passing at *input* width.
  * Symmetric norm factored: agg_d = dinv_d * sum_{s->d} dinv_s * h_s, so no
    per-edge weights; dinv folds into per-node scales.
  * Nodes relabeled + degree-packed into tiles of 128 dst slots; core c owns T
    tiles (~N/8 dsts) and the edges pointing at them. Uniform static schedule:
    every (tile, src-chunk) run padded to 512 edge slots (4 groups of 128).
  * Per layer: dma_gather (256B bf16 rows) fetches source features per edge;
    DVE builds a one-hot matrix A = (slot_id == iota) per 128-edge group and
    TensorE computes psum[dst_slot, :] += A^T @ G  (the segment sum).
  * agg -> (dinv_d scale in ACT copy) -> PE transpose -> W matmul -> bias +
    PReLU -> PE transpose back -> dinv scale -> padded bf16 store -> AllGather.
  * Mean-pool via per-tile matmuls with a host-built (1/cnt) matrix, AllReduce,
    final linear on-device.

All graph preprocessing (degrees, packing, gather index tables) is host numpy.
"""

import numpy as np
import ml_dtypes

import concourse.bacc as bacc
import concourse.mybir as mybir
import concourse.tile as tile
from concourse.bass_utils import run_bass_kernel_spmd
from concourse.library_config import mlp as mlp_lib
from concourse.masks import make_identity

F32 = mybir.dt.float32
BF16 = mybir.dt.bfloat16
I16 = mybir.dt.int16

GW = 128             # gather row width in bf16 (= 256B, dma_gather minimum)
CHUNK_ROWS = 32768   # int16 gather-index chunk size over the node space
GATHER_IDX = 1024    # max idxs per dma_gather instruction (SWDGE ring limit)
PAD_S = 255.0        # slot id for padding positions (never matches iota 0..127)


def _pos_base(t, ch, T, slots_tc, nchunk):
    """Position base of tile t, chunk ch in the per-core edge-slot list.
    Tiles are processed in batches of 64; within a batch the list is
    chunk-major, tile-minor."""
    bi = t // 64
    ntb = min(64, T - bi * 64)
    return (bi * 64 * nchunk + ch * ntb + (t - bi * 64)) * slots_tc


# ------------------------------------------------------------------ host prep
def _preprocess(x, edge_src, edge_dst, batch, n_cores, num_graphs):
    N = x.shape[0]
    src = np.concatenate([edge_src.astype(np.int64), np.arange(N, dtype=np.int64)])
    dst = np.concatenate([edge_dst.astype(np.int64), np.arange(N, dtype=np.int64)])
    deg = np.bincount(dst, minlength=N).astype(np.int64)
    dinv = (1.0 / np.sqrt(np.maximum(deg, 1))).astype(np.float32)
    core_of = (np.arange(N) % n_cores).astype(np.int64)

    cap = 1700
    while True:
        tiles_per_core = []
        for c in range(n_cores):
            nodes_c = np.where(core_of == c)[0]
            order = nodes_c[np.argsort(-deg[nodes_c], kind="stable")]
            tiles, cur, cur_deg = [], [], 0
            for v in order:
                dv = int(deg[v])
                if len(cur) >= 128 or cur_deg + dv > cap:
                    tiles.append(np.asarray(cur, dtype=np.int64))
                    cur, cur_deg = [], 0
                cur.append(v)
                cur_deg += dv
            if cur:
                tiles.append(np.asarray(cur, dtype=np.int64))
            tiles_per_core.append(tiles)
        T = max(len(t) for t in tiles_per_core)
        T += T % 2
        Npad = n_cores * T * 128
        nchunk = -(-Npad // CHUNK_ROWS)
        gid = np.full(N, -1, dtype=np.int64)
        for c in range(n_cores):
            for t, nodes in enumerate(tiles_per_core[c]):
                gid[nodes] = c * T * 128 + t * 128 + np.arange(len(nodes))
        sg, dg = gid[src], gid[dst]
        chunk_e = sg // CHUNK_ROWS
        core_e = dg // (T * 128)
        tile_e = (dg % (T * 128)) // 128
        key = ((core_e * T) + tile_e) * nchunk + chunk_e
        counts = np.bincount(key, minlength=n_cores * T * nchunk)
        gpc = -(-int(counts.max()) // 128)
        slots_tc = gpc * 128
        # total pad cost acceptable? retry with tighter cap if runs are wild
        if slots_tc * nchunk <= 2 * cap or cap < 900:
            break
        cap -= 100

    slot_e = dg % 128
    S = T * nchunk * slots_tc
    chunk_rows = CHUNK_ROWS

    order_e = np.argsort(key, kind="stable")
    sorted_key = key[order_e]
    ks = np.arange(n_cores * T * nchunk)
    run_a = np.searchsorted(sorted_key, ks)
    run_b = np.searchsorted(sorted_key, ks, side="right")

    idx_flat = np.zeros((n_cores, S), dtype=np.int16)
    s_flat = np.full((n_cores, S), PAD_S, dtype=np.float32)
    for c in range(n_cores):
        for t in range(T):
            for ch in range(nchunk):
                k = (c * T + t) * nchunk + ch
                a, b = run_a[k], run_b[k]
                if a == b:
                    continue
                es = order_e[a:b]
                base = _pos_base(t, ch, T, slots_tc, nchunk)
                idx_flat[c, base:base + (b - a)] = (sg[es] % chunk_rows).astype(np.int16)
                s_flat[c, base:base + (b - a)] = slot_e[es].astype(np.float32)

    idx_tbl = np.zeros((n_cores, 128, S // 16), dtype=np.int16)
    s_tbl = np.zeros((n_cores, 128, S // 128), dtype=ml_dtypes.bfloat16)
    for c in range(n_cores):
        idx_tbl[c] = np.tile(idx_flat[c].reshape(S // 16, 16).T, (8, 1))
        s_tbl[c] = s_flat[c].reshape(S // 128, 128).T.astype(ml_dtypes.bfloat16)

    x_perm = np.zeros((Npad, x.shape[1]), dtype=np.float32)
    x_perm[gid] = x
    dinv_all = np.ones((128, Npad // 128), dtype=np.float32)
    dinv_all[gid % 128, gid // 128] = dinv
    dinv_my = np.stack([dinv_all[:, c * T:(c + 1) * T] for c in range(n_cores)])

    cnt = np.bincount(batch, minlength=num_graphs).astype(np.float32)
    inv_cnt = (1.0 / np.maximum(cnt, 1.0)).astype(np.float32)
    M_pool = np.zeros((n_cores, 128, T * 64), dtype=np.float32)
    c_all, rem = gid // (T * 128), gid % (T * 128)
    t_all, p_all = rem // 128, rem % 128
    M_pool[c_all, p_all, t_all * 64 + batch] = inv_cnt[batch]
    M_pool = M_pool.astype(ml_dtypes.bfloat16)

    return dict(T=T, S=S, Npad=Npad, chunk_rows=chunk_rows,
                nchunk=nchunk, slots_tc=slots_tc,
                idx_tbl=idx_tbl, s_tbl=s_tbl, x_perm=x_perm,
                dinv_all=dinv_all, dinv_my=dinv_my, M_pool=M_pool)


# ------------------------------------------------------------------ device IR
def _build(meta, n_cores, IN_FEAT, widths, out_widths, num_graphs, n_classes,
           alphas):
    T, S, Npad, chunk_rows = meta["T"], meta["S"], meta["Npad"], meta["chunk_rows"]
    nchunk, slots_tc = meta["nchunk"], meta["slots_tc"]
    NL = len(widths)
    nodes_my = T * 128
    batches = [list(range(b, min(b + 64, T))) for b in range(0, T, 64)]

    nc = bacc.Bacc("TRN2", target_bir_lowering=False, debug=False,
                   num_devices=n_cores, num_swdge_queues=4)
    rg = [list(range(n_cores))]

    x_in = nc.dram_tensor("x_perm", [Npad, IN_FEAT], F32, kind="ExternalInput")
    idx_in = nc.dram_tensor("idx_tbl", [128, S // 16], I16, kind="ExternalInput")
    s_in = nc.dram_tensor("s_tbl", [128, S // 128], BF16, kind="ExternalInput")
    dinv_all_in = nc.dram_tensor("dinv_all", [128, Npad // 128], F32, kind="ExternalInput")
    dinv_my_in = nc.dram_tensor("dinv_my", [128, T], F32, kind="ExternalInput")
    M_in = nc.dram_tensor("M_pool", [128, T * 64], BF16, kind="ExternalInput")
    W_in = [nc.dram_tensor(f"W{i+1}", [widths[i], out_widths[i]], BF16,
                           kind="ExternalInput") for i in range(NL)]
    b_in = [nc.dram_tensor(f"b{i+1}", [out_widths[i], 1], F32,
                           kind="ExternalInput") for i in range(NL)]
    bn_in = [nc.dram_tensor(f"bn{i+1}", [out_widths[i], 1], F32,
                            kind="ExternalInput") for i in range(NL)]
    Wlin_in = nc.dram_tensor("Wlin", [out_widths[-1], n_classes], F32,
                             kind="ExternalInput")
    blin_in = nc.dram_tensor("blin_rep", [num_graphs, n_classes], F32,
                             kind="ExternalInput")
    out_t = nc.dram_tensor("out", [num_graphs, n_classes], F32,
                           kind="ExternalOutput")

    g = [nc.dram_tensor(f"g{i+1}", [Npad, GW], BF16) for i in range(NL)]
    h_slice = [nc.dram_tensor(f"hs{i+2}", [nodes_my, GW], BF16)
               for i in range(NL - 1)]
    pooled_d = nc.dram_tensor("pooled", [128, num_graphs], F32)
    pooled_r = nc.dram_tensor("pooled_red", [128, num_graphs], F32)

    with tile.TileContext(nc) as tc:
        with (
            tc.tile_pool(name="const", bufs=1) as cpool,
            tc.tile_pool(name="meta", bufs=2) as mpool,
            tc.tile_pool(name="gat", bufs=8) as gpool,
            tc.tile_pool(name="am", bufs=8) as apool,
            tc.tile_pool(name="big", bufs=1) as bpool,
            tc.tile_pool(name="ps", bufs=1, space="PSUM") as pspool,
        ):
            nc.gpsimd.load_library(mlp_lib)

            iden = cpool.tile([128, 128], BF16)
            make_identity(nc, iden[:])
            iota = cpool.tile([128, 128], BF16)
            nc.gpsimd.iota(iota[:], [[1, 128]], channel_multiplier=0,
                           allow_small_or_imprecise_dtypes=True)

            dinv_my = cpool.tile([128, T], F32)
            nc.sync.dma_start(dinv_my[:], dinv_my_in.ap())
            Wt, btl, bntl = [], [], []
            for i in range(NL):
                w = cpool.tile([128, out_widths[i]], BF16, tag=f"W{i}")
                nc.sync.dma_start(w[:widths[i], :], W_in[i].ap())
                Wt.append(w)
                b = cpool.tile([128, 1], F32, tag=f"b{i}")
                nc.sync.dma_start(b[:out_widths[i], :], b_in[i].ap())
                btl.append(b)
                bn = cpool.tile([128, 1], F32, tag=f"bn{i}")
                nc.sync.dma_start(bn[:out_widths[i], :], bn_in[i].ap())
                bntl.append(bn)

            # ---------------- g1 = dinv * x (full, every core)
            ncols = Npad // 128
            CCH = 1
            for cand in (28, 16, 14, 8, 7, 4, 2, 1):
                if ncols % cand == 0:
                    CCH = cand
                    break
            xv = x_in.ap().rearrange("(c p) f -> p c f", p=128)
            g1v = g[0].ap().rearrange("(c p) f -> p c f", p=128)
            for c0 in range(0, ncols, CCH):
                xt = mpool.tile([128, CCH, IN_FEAT], F32, tag="xt")
                nc.sync.dma_start(xt[:], xv[:, c0:c0 + CCH, :])
                da = mpool.tile([128, CCH], F32, tag="da")
                nc.sync.dma_start(da[:], dinv_all_in.ap()[:, c0:c0 + CCH])
                gt = mpool.tile([128, CCH, GW], BF16, tag="gt")
                nc.gpsimd.memset(gt[:], 0.0)
                nc.vector.tensor_tensor(
                    gt[:, :, :IN_FEAT], xt[:],
                    da[:, :, None].broadcast_to([128, CCH, IN_FEAT]),
                    op=mybir.AluOpType.mult)
                nc.sync.dma_start(g1v[:, c0:c0 + CCH, :], gt[:])

            gq_counter = [0]
            aggT = bpool.tile([128, nodes_my], BF16, tag="aggT")
            h_sb = bpool.tile([128, nodes_my], BF16, tag="h_sb")
            gnext = bpool.tile([128, T * GW], BF16, tag="gnext")
            agg = bpool.tile([128, T * 64], BF16, tag="agg")
            pooling_psum = None

            for li in range(NL):
                F, Fo = widths[li], out_widths[li]
                gsrc = g[li]
                # ---- aggregation
                for btiles in batches:
                    ntb = len(btiles)
                    psum = []
                    for k in range(8):
                        pst = pspool.tile([128, 512], F32, tag=f"ps{k}", name=f"pst{k}")
                        nc.vector.memset(pst[:], 0.0)
                        psum.append(pst)
                    for ch in range(nchunk):
                        base = _pos_base(btiles[0], ch, T, slots_tc, nchunk)
                        npos = ntb * slots_tc
                        idxs = mpool.tile([128, npos // 16], I16, tag="idxs")
                        nc.sync.dma_start(
                            idxs[:], idx_in.ap()[:, base // 16:(base + npos) // 16])
                        svals = mpool.tile([128, npos // 128], BF16, tag="svals")
                        nc.sync.dma_start(
                            svals[:], s_in.ap()[:, base // 128:(base + npos) // 128])
                        crows = min(chunk_rows, Npad - ch * chunk_rows)
                        srcv = gsrc.ap()[ch * chunk_rows:ch * chunk_rows + crows, :]
                        ngroups = npos // 128
                        for g0 in range(0, ngroups, 8):
                            ng = min(8, ngroups - g0)
                            nidx = ng * 128
                            gtile = gpool.tile([128, 8, GW], BF16, tag="gtile")
                            nc.gpsimd.dma_gather(
                                gtile[:, :ng, :], srcv,
                                idxs[:, g0 * 8:g0 * 8 + nidx // 16],
                                nidx, nidx, GW,
                                queue_num=gq_counter[0] % 4)
                            gq_counter[0] += 1
                            A = apool.tile([128, 8, 128], BF16, tag="A")
                            ss = svals[:, g0:g0 + ng]
                            nc.vector.tensor_tensor(
                                A[:, :ng, :],
                                ss[:, :, None].broadcast_to([128, ng, 128]),
                                iota[:, None, :].broadcast_to([128, ng, 128]),
                                op=mybir.AluOpType.is_equal)
                            for gg in range(ng):
                                pos0 = (g0 + gg) * 128
                                w = (pos0 // slots_tc) % ntb
                                last = (ch == nchunk - 1) and \
                                    (pos0 % slots_tc == slots_tc - 128)
                                nc.tensor.matmul(
                                    psum[w % 8][:, (w // 8) * 64:(w // 8) * 64 + F],
                                    A[:, gg, :], gtile[:, gg, :F],
                                    start=False, stop=last, skip_group_check=True)
                    for w, tl in enumerate(btiles):
                        nc.scalar.activation(
                            agg[:, tl * 64:tl * 64 + F],
                            psum[w % 8][:, (w // 8) * 64:(w // 8) * 64 + F],
                            mybir.ActivationFunctionType.Identity,
                            scale=dinv_my[:, tl:tl + 1])

                # ---- transpose agg -> aggT [F, nodes]
                for tl in range(T):
                    tp = pspool.tile([128, 512], BF16, tag=f"ps{tl % 2}")
                    nc.tensor.matmul(tp[:F, :128], agg[:, tl * 64:tl * 64 + F],
                                     iden[:], is_transpose=True,
                                     skip_group_check=True)
                    nc.scalar.copy(aggT[:F, tl * 128:(tl + 1) * 128], tp[:F, :128])

                # ---- h^T = W^T @ aggT + bias, PReLU
                a_f = alphas[li] if li < NL - 1 else None
                for n0 in range(0, nodes_my, 512):
                    nch = min(512, nodes_my - n0)
                    hp = pspool.tile([128, 512], F32, tag=f"ps{2 + (n0 // 512) % 2}")
                    nc.tensor.matmul(hp[:Fo, :nch], Wt[li][:F, :Fo],
                                     aggT[:F, n0:n0 + nch], skip_group_check=True)
                    if li < NL - 1:
                        # prelu(x+b) = relu(x+b) - a * relu(-x-b)
                        nc.scalar.activation(
                            h_sb[:Fo, n0:n0 + nch], hp[:Fo, :nch],
                            mybir.ActivationFunctionType.Relu,
                            bias=btl[li][:Fo, :], scale=1.0)
                        hrelu = mpool.tile([128, 512], BF16, tag="hrelu")
                        nc.scalar.activation(
                            hrelu[:Fo, :nch], hp[:Fo, :nch],
                            mybir.ActivationFunctionType.Relu,
                            bias=bntl[li][:Fo, :], scale=-1.0)
                        nc.vector.scalar_tensor_tensor(
                            h_sb[:Fo, n0:n0 + nch], hrelu[:Fo, :nch],
                            float(-a_f), h_sb[:Fo, n0:n0 + nch],
                            op0=mybir.AluOpType.mult, op1=mybir.AluOpType.add)
                    else:
                        nc.scalar.activation(
                            h_sb[:Fo, n0:n0 + nch], hp[:Fo, :nch],
                            mybir.ActivationFunctionType.Identity,
                            bias=btl[li][:Fo, :], scale=1.0)

                # ---- transpose back; dinv-scale (layers 1-3) or pooling (L4)
                if li < NL - 1:
                    nc.gpsimd.memset(gnext[:], 0.0)
                for tl in range(T):
                    tp2 = pspool.tile([128, 512], BF16, tag=f"ps{4 + tl % 2}")
                    nc.tensor.matmul(tp2[:128, :Fo],
                                     h_sb[:Fo, tl * 128:(tl + 1) * 128],
                                     iden[:Fo, :Fo], is_transpose=True,
                                     skip_group_check=True)
                    if li < NL - 1:
                        nc.scalar.activation(
                            gnext[:, tl * GW:tl * GW + Fo], tp2[:, :Fo],
                            mybir.ActivationFunctionType.Identity,
                            scale=dinv_my[:, tl:tl + 1])
                    else:
                        h4n = mpool.tile([128, 128], BF16, tag="h4n")
                        nc.vector.tensor_copy(h4n[:, :Fo], tp2[:, :Fo])
                        if pooling_psum is None:
                            Mall = bpool.tile([128, T * 64], BF16, tag="Mall")
                            nc.sync.dma_start(Mall[:], M_in.ap())
                            pooling_psum = pspool.tile([128, 512], F32, tag="ps6")
                        nc.tensor.matmul(
                            pooling_psum[:Fo, :num_graphs], h4n[:, :Fo],
                            Mall[:, tl * 64:tl * 64 + num_graphs], start=(tl == 0),
                            stop=(tl == T - 1), skip_group_check=True)

                if li < NL - 1:
                    hsv = h_slice[li].ap().rearrange("(t p) f -> p t f", p=128)
                    nc.sync.dma_start(
                        hsv[:], gnext[:].rearrange("p (t f) -> p t f", f=GW))
                    if n_cores > 1:
                        nc.gpsimd.collective_compute(
                            "AllGather", mybir.AluOpType.bypass, rg,
                            [h_slice[li].ap()], [g[li + 1].ap()])
                    else:
                        nc.sync.dma_start(g[li + 1].ap()[:nodes_my, :],
                                          h_slice[li].ap())

            # ---------------- pooled -> AllReduce -> final linear
            Fo = out_widths[-1]
            pooled_sb = cpool.tile([128, num_graphs], F32, tag="pooled")
            nc.vector.tensor_copy(pooled_sb[:Fo, :], pooling_psum[:Fo, :num_graphs])
            if n_cores > 1:
                nc.sync.dma_start(pooled_d.ap()[:Fo, :], pooled_sb[:Fo, :])
                nc.gpsimd.collective_compute(
                    "AllReduce", mybir.AluOpType.add, rg,
                    [pooled_d.ap()], [pooled_r.ap()])
                pooled2 = cpool.tile([128, num_graphs], F32, tag="pooled2")
                nc.sync.dma_start(pooled2[:Fo, :], pooled_r.ap()[:Fo, :])
            else:
                pooled2 = pooled_sb
            Wlin_sb = cpool.tile([128, n_classes], F32, tag="wlin")
            nc.sync.dma_start(Wlin_sb[:Fo, :], Wlin_in.ap())
            blin_sb = cpool.tile([num_graphs, n_classes], F32, tag="blin")
            nc.sync.dma_start(blin_sb[:], blin_in.ap())
            fin = pspool.tile([128, 512], F32, tag="ps7")
            nc.tensor.matmul(fin[:num_graphs, :n_classes], pooled2[:Fo, :num_graphs],
                             Wlin_sb[:Fo, :], skip_group_check=True)
            out_sb = cpool.tile([num_graphs, n_classes], F32, tag="outsb")
            nc.vector.tensor_tensor(out_sb[:], fin[:num_graphs, :n_classes],
                                    blin_sb[:], op=mybir.AluOpType.add)
            nc.sync.dma_start(out_t.ap(), out_sb[:])

    nc.compile()
    return nc


# ------------------------------------------------------------------ entry
def kernel(x, edge_src, edge_dst, batch,
           W1, b1, W2, b2, W3, b3, W4, b4,
           a1, a2, a3, Wlin, blin, n_cores=8):
    x = np.asarray(x, dtype=np.float32)
    edge_src = np.asarray(edge_src, dtype=np.int32)
    edge_dst = np.asarray(edge_dst, dtype=np.int32)
    batch = np.asarray(batch, dtype=np.int32)
    Ws = [np.asarray(w, np.float32) for w in (W1, W2, W3, W4)]
    bs = [np.asarray(b, np.float32) for b in (b1, b2, b3, b4)]
    alphas = [float(a1), float(a2), float(a3)]
    Wlin = np.asarray(Wlin, np.float32)
    blin = np.asarray(blin, np.float32)

    IN_FEAT = x.shape[1]
    widths = [IN_FEAT] + [w.shape[1] for w in Ws[:-1]]
    out_widths = [w.shape[1] for w in Ws]
    NG = 64
    NCLS = Wlin.shape[1]

    meta = _preprocess(x, edge_src, edge_dst, batch, n_cores, NG)
    nc = _build(meta, n_cores, IN_FEAT, widths, out_widths, NG, NCLS, alphas)

    in_maps = []
    for c in range(n_cores):
        m = dict(
            x_perm=meta["x_perm"],
            idx_tbl=meta["idx_tbl"][c],
            s_tbl=np.asarray(meta["s_tbl"][c]),
            dinv_all=meta["dinv_all"],
            dinv_my=np.ascontiguousarray(meta["dinv_my"][c]),
            M_pool=np.asarray(meta["M_pool"][c]),
            Wlin=Wlin,
            blin_rep=np.tile(blin[None, :], (NG, 1)).astype(np.float32),
        )
        for i in range(4):
            m[f"W{i+1}"] = Ws[i].astype(ml_dtypes.bfloat16)
            m[f"b{i+1}"] = np.ascontiguousarray(bs[i].reshape(-1, 1))
            m[f"bn{i+1}"] = np.ascontiguousarray(-bs[i].reshape(-1, 1))
        in_maps.append(m)

    res = run_bass_kernel_spmd(nc, in_maps, core_ids=list(range(n_cores)))
    return np.asarray(res.results[0]["out"], dtype=np.float32)



# revision 2
# speedup vs baseline: 2.3657x; 2.3657x over previous
"""GCN forward (4-layer GCNConv + global mean-pool + linear) on 8 TRN2 cores.

Strategy (graph/dst-node data parallelism per the sharding hint):
  * Associativity: S @ (h W) == (S @ h) W  -> message passing at *input* width.
  * Symmetric norm factored: agg_d = dinv_d * (sum_{s->d} dinv_s * h_s + dinv_d
    * h_d); self-loop term is added on-chip from the resident own-shard tile,
    so self-loops never enter the gather tables.
  * Nodes dealt round-robin (by descending in-degree) into tiles of <=128 dst
    slots per core; edges bucketed by (tile, src-chunk of 32768 rows).  Group
    counts per bucket are ceil(max-over-cores/128) so the single SPMD program
    fits every core with minimal padding.
  * Per layer: dma_gather (256B bf16 rows) fetches source features per edge;
    DVE builds a one-hot matrix A = (slot_id == iota) per 128-edge group and
    TensorE computes psum[dst_slot, :] += A^T @ G  (the segment sum).
  * agg+self -> (dinv_d scale) -> PE transpose -> W matmul -> bias + PReLU ->
    PE transpose back -> dinv scale -> bf16 store -> AllGather.
  * Mean-pool via per-tile one-hot (graph-id == iota) matmuls, AllReduce,
    1/cnt as a per-partition scale on the final linear.

All inputs ship as ONE flat uint8 tensor per core (~1.1 MB); on-device bitcast
views slice out the packed sections.  The 8x partition replication the SWDGE
gather ucode needs for its index table is done with DRAM->DRAM copies on
device instead of on host.
"""

import numpy as np
import ml_dtypes

import concourse.bacc as bacc
import concourse.mybir as mybir
import concourse.tile as tile
from concourse.bass_utils import run_bass_kernel_spmd
from concourse.library_config import mlp as mlp_lib
from concourse.masks import make_identity

F32 = mybir.dt.float32
BF16 = mybir.dt.bfloat16
I16 = mybir.dt.int16
U8 = mybir.dt.uint8

GW = 128             # gather row width in bf16 (= 256B, dma_gather minimum)
CHUNK_ROWS = 32768   # int16 gather-index reach over the node space
PAD_S = 255          # slot id for padding positions (never matches iota 0..127)
N_CORES = 8
NUM_GRAPHS = 64


def _align(x, a=512):
    return (x + a - 1) // a * a


# ------------------------------------------------------------------ host prep
def _preprocess(x, edge_src, edge_dst, batch, n_cores, num_graphs):
    N = x.shape[0]
    IN_FEAT = x.shape[1]
    src = edge_src.astype(np.int64)
    dst = edge_dst.astype(np.int64)
    indeg = np.bincount(dst, minlength=N).astype(np.int64)   # w/o self-loop
    deg = indeg + 1                                          # with self-loop
    dinv = (1.0 / np.sqrt(deg)).astype(np.float32)
    core_of = (np.arange(N) % n_cores).astype(np.int64)

    # deal nodes round-robin by descending in-degree into T tiles per core
    T = 132
    gid = np.full(N, -1, dtype=np.int64)
    for c in range(n_cores):
        nodes_c = np.where(core_of == c)[0]
        order = nodes_c[np.argsort(-indeg[nodes_c], kind="stable")]
        r = np.arange(len(order))
        gid[order] = c * T * 128 + (r % T) * 128 + (r // T)
    assert gid[gid >= 0].max() < n_cores * T * 128
    Npad = n_cores * T * 128
    nchunk = -(-Npad // CHUNK_ROWS)

    sg, dg = gid[src], gid[dst]
    core_e = dg // (T * 128)
    tile_e = (dg % (T * 128)) // 128
    slot_e = dg % 128
    ch_e = sg // CHUNK_ROWS
    key = (core_e * T + tile_e) * nchunk + ch_e
    cnt = np.bincount(key, minlength=n_cores * T * nchunk)\
        .reshape(n_cores, T, nchunk)
    gmax = -(-cnt.max(axis=0) // 128)                         # [T, nchunk]

    # schedule: batches of 64 tiles; within batch iterate chunk, tile, groups
    batches = [list(range(b, min(b + 64, T))) for b in range(0, T, 64)]
    base = np.zeros((T, nchunk), dtype=np.int64)              # 128-group index
    sched = []                                                # [b][ch] -> (pos0, [(w, last)])
    last_cell = {}
    for t in range(T):
        nz = np.where(gmax[t] > 0)[0]
        if len(nz):
            last_cell[t] = nz[-1]
    pos = 0
    for bi, btiles in enumerate(batches):
        per_ch = []
        for ch in range(nchunk):
            pos0 = pos
            groups = []
            for w, t in enumerate(btiles):
                g = int(gmax[t, ch])
                base[t, ch] = pos
                for k in range(g):
                    last = (ch == last_cell.get(t)) and (k == g - 1)
                    groups.append((w, last))
                pos += g
            per_ch.append((pos0, groups))
        sched.append(per_ch)
    S = pos * 128
    assert S % 128 == 0

    # per-edge positions: sort by key, offset within run, add cell base
    order_e = np.argsort(key, kind="stable")
    sorted_key = key[order_e]
    run_start = np.searchsorted(sorted_key, sorted_key)       # first idx of run
    off_in_run = np.arange(len(order_e)) - run_start
    cell_base = base[tile_e[order_e], ch_e[order_e]] * 128
    pos_e = cell_base + off_in_run                            # per-core position

    idx_flat = np.zeros((n_cores, S), dtype=np.int16)
    s_flat = np.full((n_cores, S), PAD_S, dtype=np.uint8)
    ce = core_e[order_e]
    idx_flat[ce, pos_e] = (sg[order_e] % CHUNK_ROWS).astype(np.int16)
    s_flat[ce, pos_e] = slot_e[order_e].astype(np.uint8)

    idx_tbl = np.ascontiguousarray(
        idx_flat.reshape(n_cores, S // 16, 16).transpose(0, 2, 1))  # [C,16,S/16]
    s_tbl = np.ascontiguousarray(
        s_flat.reshape(n_cores, S // 128, 128).transpose(0, 2, 1))  # [C,128,S/128]

    # node-slot tables [C, 128, T]
    p_all = gid % 128
    t_all = (gid % (T * 128)) // 128
    c_all = gid // (T * 128)
    x_bf = np.zeros((n_cores, 128, T, IN_FEAT), dtype=ml_dtypes.bfloat16)
    x_bf[c_all, p_all, t_all] = x.astype(ml_dtypes.bfloat16)
    dinv_my = np.zeros((n_cores, 128, T), dtype=np.float32)
    dinv_my[c_all, p_all, t_all] = dinv
    bgid = np.full((n_cores, 128, T), 255.0, dtype=np.float32)
    bgid[c_all, p_all, t_all] = batch.astype(np.float32)

    cnt_g = np.bincount(batch, minlength=num_graphs).astype(np.float32)
    inv_cnt = (1.0 / np.maximum(cnt_g, 1.0)).astype(np.float32)

    return dict(T=T, S=S, Npad=Npad, nchunk=nchunk, sched=sched,
                idx_tbl=idx_tbl, s_tbl=s_tbl, x_bf=x_bf,
                dinv_my=dinv_my, bgid=bgid, inv_cnt=inv_cnt)


def _pack_mega(meta, core, Ws, bs, Wlin, blin, IN_FEAT, out_widths):
    """Assemble the single flat uint8 input for one core."""
    T, S = meta["T"], meta["S"]
    # pack_f32 [128, PCOLS]: x(bf16->4T f32) | dinv(T) | bgid(T) | W(bf16->120)
    #                        | b,bn(8) | Wlin(4) | blin(4) | inv_cnt(1)
    xcols = IN_FEAT * T // 2
    PCOLS = xcols + T + T + 120 + 8 + 4 + 4 + 1
    pf = np.zeros((128, PCOLS), dtype=np.float32)
    xb = np.ascontiguousarray(
        meta["x_bf"][core].reshape(128, T * IN_FEAT))         # [128, 8T] bf16
    pf[:, :xcols] = xb.view(np.float32)
    o = xcols
    pf[:, o:o + T] = meta["dinv_my"][core]; o += T
    pf[:, o:o + T] = meta["bgid"][core]; o += T
    wp = np.zeros((128, 240), dtype=ml_dtypes.bfloat16)
    woff = 0
    for w in Ws:
        fi, fo = w.shape
        wp[:fi, woff:woff + fo] = w.astype(ml_dtypes.bfloat16)
        woff += fo
    pf[:, o:o + 120] = wp.view(np.float32); o += 120
    for i in range(4):
        pf[:out_widths[i], o] = bs[i]; o += 1
    for i in range(4):
        pf[:out_widths[i], o] = -bs[i]; o += 1
    pf[:Wlin.shape[0], o:o + 4] = Wlin; o += 4
    pf[:, o:o + 4] = blin[None, :]; o += 4
    pf[:NUM_GRAPHS, o] = meta["inv_cnt"]; o += 1
    assert o == PCOLS

    sz_pf = 128 * PCOLS * 4
    sz_idx = S * 2
    sz_s = S
    off_idx = _align(sz_pf)
    off_s = _align(off_idx + sz_idx)
    nb = _align(off_s + sz_s)
    mega = np.zeros(nb, dtype=np.uint8)
    mega[:sz_pf] = np.frombuffer(pf.tobytes(), np.uint8)
    mega[off_idx:off_idx + sz_idx] = np.frombuffer(
        meta["idx_tbl"][core].tobytes(), np.uint8)
    mega[off_s:off_s + sz_s] = np.frombuffer(
        meta["s_tbl"][core].tobytes(), np.uint8)
    layout = dict(PCOLS=PCOLS, xcols=xcols, off_idx=off_idx, off_s=off_s, nb=nb)
    return mega, layout


# ------------------------------------------------------------------ device IR
def _build(meta, layout, n_cores, IN_FEAT, widths, out_widths, num_graphs,
           n_classes, alphas):
    T, S, Npad, nchunk = meta["T"], meta["S"], meta["Npad"], meta["nchunk"]
    sched = meta["sched"]
    NL = len(widths)
    nodes_my = T * 128
    batches = [list(range(b, min(b + 64, T))) for b in range(0, T, 64)]
    PCOLS, xcols = layout["PCOLS"], layout["xcols"]
    GMAX = max(len(g) for per_ch in sched for (_, g) in per_ch)

    nc = bacc.Bacc("TRN2", target_bir_lowering=False, debug=False,
                   num_devices=n_cores, num_swdge_queues=4)
    rg = [list(range(n_cores))]

    mega = nc.dram_tensor("mega", [layout["nb"]], U8, kind="ExternalInput")
    out_t = nc.dram_tensor("out", [num_graphs, n_classes], F32,
                           kind="ExternalOutput")

    idx_full = nc.dram_tensor("idx_full", [128, S // 16], I16)
    s_full = nc.dram_tensor("s_full", [128, S // 128], BF16)
    g = [nc.dram_tensor(f"g{i+1}", [Npad, GW], BF16) for i in range(NL)]
    h_slice = [nc.dram_tensor(f"hs{i+1}", [nodes_my, GW], BF16)
               for i in range(NL)]
    pooled_d = nc.dram_tensor("pooled", [128, num_graphs], F32)
    pooled_r = nc.dram_tensor("pooled_red", [128, num_graphs], F32)

    with tile.TileContext(nc) as tc:
        with (
            tc.tile_pool(name="const", bufs=1) as cpool,
            tc.tile_pool(name="meta", bufs=2) as mpool,
            tc.tile_pool(name="gat", bufs=8) as gpool,
            tc.tile_pool(name="am", bufs=8) as apool,
            tc.tile_pool(name="big", bufs=1) as bpool,
            tc.tile_pool(name="ps", bufs=1, space="PSUM") as pspool,
        ):
            nc.gpsimd.load_library(mlp_lib)

            iden = cpool.tile([128, 128], BF16)
            make_identity(nc, iden[:])
            iota = cpool.tile([128, 128], BF16)
            nc.gpsimd.iota(iota[:], [[1, 128]], channel_multiplier=0,
                           allow_small_or_imprecise_dtypes=True)

            # ---------------- unpack mega
            pf = cpool.tile([128, PCOLS], F32, tag="pf")
            nc.sync.dma_start(
                pf[:], mega.ap()[:128 * PCOLS * 4].bitcast(F32)
                .rearrange("(p c) -> p c", p=128))
            o = xcols
            dinv_my = pf[:, o:o + T]; o += T
            bgid_f = pf[:, o:o + T]; o += T
            wcols = pf[:, o:o + 120]; o += 120
            btl = [pf[:, o + i:o + i + 1] for i in range(NL)]; o += NL
            bntl = [pf[:, o + i:o + i + 1] for i in range(NL)]; o += NL
            Wlin_sb = pf[:, o:o + n_classes]; o += n_classes
            blin_sb = pf[:, o:o + n_classes]; o += n_classes
            inv_cnt = pf[:, o:o + 1]; o += 1

            xt = cpool.tile([128, T * IN_FEAT], BF16, tag="xt")
            nc.vector.tensor_copy(xt[:], pf[:, 0:xcols].bitcast(BF16))
            wt = cpool.tile([128, 240], BF16, tag="wt")
            nc.vector.tensor_copy(wt[:], wcols.bitcast(BF16))
            woffs = np.cumsum([0] + out_widths).tolist()
            Wt = [wt[:, woffs[i]:woffs[i + 1]] for i in range(NL)]
            bgid = cpool.tile([128, T], BF16, tag="bgid")
            nc.vector.tensor_copy(bgid[:], bgid_f)

            # ---------------- replicate idx table (16 -> 128 partitions)
            idx_view = mega.ap()[layout["off_idx"]:layout["off_idx"] + S * 2]\
                .bitcast(I16).rearrange("(p c) -> p c", p=16)
            for k in range(8):
                nc.sync.dma_start(idx_full.ap()[16 * k:16 * (k + 1), :],
                                  idx_view)
            # ---------------- slot table uint8 -> bf16
            su = mpool.tile([128, S // 128], U8, tag="su")
            nc.sync.dma_start(
                su[:], mega.ap()[layout["off_s"]:layout["off_s"] + S]
                .rearrange("(p c) -> p c", p=128))
            sb = mpool.tile([128, S // 128], BF16, tag="sb")
            nc.vector.tensor_copy(sb[:], su[:])
            nc.sync.dma_start(s_full.ap(), sb[:])

            # ---------------- g1 own shard = dinv * x, AllGather
            gnext = bpool.tile([128, T * GW], BF16, tag="gnext")
            aggT = bpool.tile([128, nodes_my], BF16, tag="aggT")
            h_sb = bpool.tile([128, nodes_my], BF16, tag="h_sb")
            agg = bpool.tile([128, T * 64], BF16, tag="agg")

            nc.gpsimd.memset(gnext[:], 0.0)
            g3 = gnext[:].rearrange("p (t f) -> p t f", f=GW)
            x3 = xt[:].rearrange("p (t f) -> p t f", f=IN_FEAT)
            nc.vector.tensor_tensor(
                g3[:, :, :IN_FEAT], x3,
                dinv_my[:, :, None].broadcast_to([128, T, IN_FEAT]),
                op=mybir.AluOpType.mult)
            hsv = h_slice[0].ap().rearrange("(t p) f -> p t f", p=128)
            nc.sync.dma_start(hsv[:], g3)
            if n_cores > 1:
                nc.gpsimd.collective_compute(
                    "AllGather", mybir.AluOpType.bypass, rg,
                    [h_slice[0].ap()], [g[0].ap()])
            else:
                nc.sync.dma_start(g[0].ap()[:nodes_my, :], h_slice[0].ap())

            gq_counter = [0]
            pooling_psum = None

            for li in range(NL):
                F, Fo = widths[li], out_widths[li]
                gsrc = g[li]
                # ---- aggregation
                for bi, btiles in enumerate(batches):
                    psum = []
                    for k in range(8):
                        pst = pspool.tile([128, 512], F32, tag=f"ps{k}",
                                          name=f"pst{k}")
                        nc.vector.memset(pst[:], 0.0)
                        psum.append(pst)
                    for ch in range(nchunk):
                        pos0, groups = sched[bi][ch]
                        ngr = len(groups)
                        if ngr == 0:
                            continue
                        idxs = mpool.tile([128, GMAX * 8], I16, tag="idxs")
                        nc.sync.dma_start(
                            idxs[:, :ngr * 8],
                            idx_full.ap()[:, pos0 * 8:(pos0 + ngr) * 8])
                        svals = mpool.tile([128, GMAX], BF16, tag="svals")
                        nc.sync.dma_start(
                            svals[:, :ngr], s_full.ap()[:, pos0:pos0 + ngr])
                        crows = min(CHUNK_ROWS, Npad - ch * CHUNK_ROWS)
                        srcv = gsrc.ap()[ch * CHUNK_ROWS:
                                         ch * CHUNK_ROWS + crows, :]
                        for g0 in range(0, ngr, 8):
                            ng = min(8, ngr - g0)
                            nidx = ng * 128
                            gtile = gpool.tile([128, 8, GW], BF16, tag="gtile")
                            nc.gpsimd.dma_gather(
                                gtile[:, :ng, :], srcv,
                                idxs[:, g0 * 8:g0 * 8 + nidx // 16],
                                nidx, nidx, GW,
                                queue_num=gq_counter[0] % 4)
                            gq_counter[0] += 1
                            A = apool.tile([128, 8, 128], BF16, tag="A")
                            ss = svals[:, g0:g0 + ng]
                            nc.vector.tensor_tensor(
                                A[:, :ng, :],
                                ss[:, :, None].broadcast_to([128, ng, 128]),
                                iota[:, None, :].broadcast_to([128, ng, 128]),
                                op=mybir.AluOpType.is_equal)
                            for gg in range(ng):
                                w, last = groups[g0 + gg]
                                nc.tensor.matmul(
                                    psum[w % 8][:, (w // 8) * 64:
                                                (w // 8) * 64 + F],
                                    A[:, gg, :], gtile[:, gg, :F],
                                    start=False, stop=last,
                                    skip_group_check=True)
                    # self-loop term + dinv_d scale
                    for w, tl in enumerate(btiles):
                        ps_sl = psum[w % 8][:, (w // 8) * 64:(w // 8) * 64 + F]
                        nc.vector.tensor_tensor(
                            ps_sl, ps_sl, gnext[:, tl * GW:tl * GW + F],
                            op=mybir.AluOpType.add)
                        nc.scalar.activation(
                            agg[:, tl * 64:tl * 64 + F], ps_sl,
                            mybir.ActivationFunctionType.Identity,
                            scale=dinv_my[:, tl:tl + 1])

                # ---- transpose agg -> aggT [F, nodes]
                for tl in range(T):
                    tp = pspool.tile([128, 512], BF16, tag=f"ps{tl % 2}")
                    nc.tensor.matmul(tp[:F, :128], agg[:, tl * 64:tl * 64 + F],
                                     iden[:], is_transpose=True,
                                     skip_group_check=True)
                    nc.scalar.copy(aggT[:F, tl * 128:(tl + 1) * 128],
                                   tp[:F, :128])

                # ---- h^T = W^T @ aggT + bias, PReLU
                a_f = alphas[li] if li < NL - 1 else None
                for n0 in range(0, nodes_my, 512):
                    nch = min(512, nodes_my - n0)
                    hp = pspool.tile([128, 512], F32,
                                     tag=f"ps{2 + (n0 // 512) % 2}")
                    nc.tensor.matmul(hp[:Fo, :nch], Wt[li][:F, :Fo],
                                     aggT[:F, n0:n0 + nch],
                                     skip_group_check=True)
                    if li < NL - 1:
                        # prelu(x+b) = relu(x+b) - a * relu(-x-b)
                        nc.scalar.activation(
                            h_sb[:Fo, n0:n0 + nch], hp[:Fo, :nch],
                            mybir.ActivationFunctionType.Relu,
                            bias=btl[li][:Fo, :], scale=1.0)
                        hrelu = mpool.tile([128, 512], BF16, tag="hrelu")
                        nc.scalar.activation(
                            hrelu[:Fo, :nch], hp[:Fo, :nch],
                            mybir.ActivationFunctionType.Relu,
                            bias=bntl[li][:Fo, :], scale=-1.0)
                        nc.vector.scalar_tensor_tensor(
                            h_sb[:Fo, n0:n0 + nch], hrelu[:Fo, :nch],
                            float(-a_f), h_sb[:Fo, n0:n0 + nch],
                            op0=mybir.AluOpType.mult, op1=mybir.AluOpType.add)
                    else:
                        nc.scalar.activation(
                            h_sb[:Fo, n0:n0 + nch], hp[:Fo, :nch],
                            mybir.ActivationFunctionType.Identity,
                            bias=btl[li][:Fo, :], scale=1.0)

                # ---- transpose back; dinv-scale (layers 1-3) or pooling (L4)
                if li < NL - 1:
                    nc.gpsimd.memset(gnext[:], 0.0)
                for tl in range(T):
                    tp2 = pspool.tile([128, 512], BF16, tag=f"ps{4 + tl % 2}")
                    nc.tensor.matmul(tp2[:128, :Fo],
                                     h_sb[:Fo, tl * 128:(tl + 1) * 128],
                                     iden[:Fo, :Fo], is_transpose=True,
                                     skip_group_check=True)
                    if li < NL - 1:
                        nc.scalar.activation(
                            gnext[:, tl * GW:tl * GW + Fo], tp2[:, :Fo],
                            mybir.ActivationFunctionType.Identity,
                            scale=dinv_my[:, tl:tl + 1])
                    else:
                        h4n = mpool.tile([128, 128], BF16, tag="h4n")
                        nc.vector.tensor_copy(h4n[:, :Fo], tp2[:, :Fo])
                        oh = apool.tile([128, 64], BF16, tag="oh")
                        nc.vector.tensor_tensor(
                            oh[:],
                            bgid[:, tl:tl + 1].broadcast_to([128, 64]),
                            iota[:, :64], op=mybir.AluOpType.is_equal)
                        if pooling_psum is None:
                            pooling_psum = pspool.tile([128, 512], F32,
                                                       tag="ps6")
                        nc.tensor.matmul(
                            pooling_psum[:Fo, :num_graphs], h4n[:, :Fo],
                            oh[:], start=(tl == 0), stop=(tl == T - 1),
                            skip_group_check=True)

                if li < NL - 1:
                    hsv = h_slice[li + 1].ap().rearrange("(t p) f -> p t f",
                                                         p=128)
                    nc.sync.dma_start(
                        hsv[:], gnext[:].rearrange("p (t f) -> p t f", f=GW))
                    if n_cores > 1:
                        nc.gpsimd.collective_compute(
                            "AllGather", mybir.AluOpType.bypass, rg,
                            [h_slice[li + 1].ap()], [g[li + 1].ap()])
                    else:
                        nc.sync.dma_start(g[li + 1].ap()[:nodes_my, :],
                                          h_slice[li + 1].ap())

            # ---------------- pooled -> AllReduce -> final linear
            Fo = out_widths[-1]
            pooled_sb = cpool.tile([128, num_graphs], F32, tag="pooled")
            nc.vector.tensor_copy(pooled_sb[:Fo, :],
                                  pooling_psum[:Fo, :num_graphs])
            if n_cores > 1:
                nc.sync.dma_start(pooled_d.ap()[:Fo, :], pooled_sb[:Fo, :])
                nc.gpsimd.collective_compute(
                    "AllReduce", mybir.AluOpType.add, rg,
                    [pooled_d.ap()], [pooled_r.ap()])
                pooled2 = cpool.tile([128, num_graphs], F32, tag="pooled2")
                nc.sync.dma_start(pooled2[:Fo, :], pooled_r.ap()[:Fo, :])
            else:
                pooled2 = pooled_sb
            fin = pspool.tile([128, 512], F32, tag="ps7")
            nc.tensor.matmul(fin[:num_graphs, :n_classes],
                             pooled2[:Fo, :num_graphs],
                             Wlin_sb[:Fo, :], skip_group_check=True)
            out_sb = cpool.tile([num_graphs, n_classes], F32, tag="outsb")
            nc.scalar.activation(
                out_sb[:], fin[:num_graphs, :n_classes],
                mybir.ActivationFunctionType.Identity,
                scale=inv_cnt[:num_graphs, :])
            nc.vector.tensor_tensor(out_sb[:], out_sb[:],
                                    blin_sb[:num_graphs, :],
                                    op=mybir.AluOpType.add)
            nc.sync.dma_start(out_t.ap(), out_sb[:])

    nc.compile()
    return nc


def _make_in_maps(meta, inputs, n_cores):
    Ws = [np.asarray(inputs[f"W{i+1}"], np.float32) for i in range(4)]
    bs = [np.asarray(inputs[f"b{i+1}"], np.float32) for i in range(4)]
    Wlin = np.asarray(inputs["Wlin"], np.float32)
    blin = np.asarray(inputs["blin"], np.float32)
    out_widths = [w.shape[1] for w in Ws]
    in_maps, layout = [], None
    for c in range(n_cores):
        mega, layout = _pack_mega(meta, c, Ws, bs, Wlin, blin,
                                  int(inputs["x"].shape[1]), out_widths)
        in_maps.append(dict(mega=mega))
    return in_maps, layout


# ------------------------------------------------------------------ entry
def kernel(x, edge_src, edge_dst, batch,
           W1, b1, W2, b2, W3, b3, W4, b4,
           a1, a2, a3, Wlin, blin, n_cores=N_CORES):
    x = np.asarray(x, dtype=np.float32)
    edge_src = np.asarray(edge_src, dtype=np.int32)
    edge_dst = np.asarray(edge_dst, dtype=np.int32)
    batch = np.asarray(batch, dtype=np.int32)
    Ws = [np.asarray(w, np.float32) for w in (W1, W2, W3, W4)]
    alphas = [float(a1), float(a2), float(a3)]
    Wlin = np.asarray(Wlin, np.float32)
    blin = np.asarray(blin, np.float32)

    IN_FEAT = x.shape[1]
    widths = [IN_FEAT] + [w.shape[1] for w in Ws[:-1]]
    out_widths = [w.shape[1] for w in Ws]
    NCLS = Wlin.shape[1]

    meta = _preprocess(x, edge_src, edge_dst, batch, n_cores, NUM_GRAPHS)
    inputs = dict(x=x, W1=Ws[0], b1=b1, W2=Ws[1], b2=b2, W3=Ws[2], b3=b3,
                  W4=Ws[3], b4=b4, Wlin=Wlin, blin=blin)
    in_maps, layout = _make_in_maps(meta, inputs, n_cores)
    nc = _build(meta, layout, n_cores, IN_FEAT, widths, out_widths,
                NUM_GRAPHS, NCLS, alphas)
    res = run_bass_kernel_spmd(nc, in_maps, core_ids=list(range(n_cores)))
    return np.asarray(res.results[0]["out"], dtype=np.float32)
